# revision 1
# baseline (speedup 1.0000x reference)
"""Single-head causal attention (B=4, N=2048, D=1024, fp32) on 8 TRN2 cores.

Sharding: 8 cores = (batch b in 0..3) x (sequence half s in 0..1), one SPMD
program. Each core projects K,V for all 2048 keys of its batch (duplicated
across the pair), projects Q for its 1024 queries, and runs the causal
softmax(Q K^T / sqrt(dk)) @ V for its 8 query blocks of 128. All per-core
variation (which queries / which causal masks) is carried in host-prepared
input data, so the single program serves all cores.

Device layouts are host-pretransposed so every matmul contraction dim lands
on SBUF partitions. Matmuls run in bf16 with fp32 PSUM accumulation.
"""
import numpy as np
import ml_dtypes

import concourse.bass as bass
import concourse.mybir as mybir
from concourse.tile import TileContext
from concourse.masks import make_identity
from concourse.bass_utils import run_bass_kernel_spmd

F32 = mybir.dt.float32
BF16 = mybir.dt.bfloat16

B = 4
N = 2048
D = 1024
NK = 2048
NQ = 1024
DV = 1024
NB = 8          # q-blocks per core
P = 128
C = 512         # psum chunk width
SCALE = 1.0 / 32.0   # 1/sqrt(dk)


def _split_multi_waits(nc):
    """walrus in this container rejects >1 sync-wait per instruction; hoist
    extra waits onto same-engine nops placed immediately before."""
    eng = {
        mybir.EngineType.PE: "tensor",
        mybir.EngineType.Activation: "scalar",
        mybir.EngineType.DVE: "vector",
        mybir.EngineType.Pool: "gpsimd",
        mybir.EngineType.SP: "sync",
    }
    blocks = list(nc.m.functions[0].blocks)
    snapshots = [(b, list(b.instructions)) for b in blocks]
    new_lists = []
    for b, insts in snapshots:
        new_list = []
        for inst in insts:
            si = inst.sync_info
            waits = list(si.on_wait) if si and si.on_wait else []
            if len(waits) > 1:
                si.on_wait = waits[-1:]
                for w in waits[:-1]:
                    nop = getattr(nc, eng[inst.engine]).nop().ins
                    nsi = nop.sync_info
                    if nsi is None:
                        nop.sync_info = mybir.SyncInfo(on_wait=[w], on_update=[])
                    else:
                        nsi.on_wait = [w]
                        nsi.on_update = []
                    new_list.append(nop)
            new_list.append(inst)
        new_lists.append((b, new_list))
    for b, new_list in new_lists:
        b.instructions = new_list


def _build(DT=BF16):
    nc = bass.Bass("TRN2", target_bir_lowering=False, debug=False, num_devices=8)

    xkv_d = nc.dram_tensor("xkv", [D, NK], DT, kind="ExternalInput").ap()
    xq_d = nc.dram_tensor("xq", [D, NQ], DT, kind="ExternalInput").ap()
    wq_d = nc.dram_tensor("wq", [D, 1024], DT, kind="ExternalInput").ap()
    wk_d = nc.dram_tensor("wk", [D, 1024], DT, kind="ExternalInput").ap()
    wv_d = nc.dram_tensor("wv", [D, 1024], DT, kind="ExternalInput").ap()
    mask_d = nc.dram_tensor("masks", [NB, P, NK], F32, kind="ExternalInput").ap()
    y_d = nc.dram_tensor("y", [NB, P, DV], F32, kind="ExternalOutput").ap()

    with TileContext(nc) as tc:
        with tc.tile_pool(name="qkv", bufs=1) as qkv:
            QT = [qkv.tile([P, NQ], DT, tag=f"qt{i}", name=f"qt{i}") for i in range(8)]
            KT = [qkv.tile([P, NK], DT, tag=f"kt{i}", name=f"kt{i}") for i in range(8)]
            V = [qkv.tile([P, DV], DT, tag=f"v{i}", name=f"v{i}") for i in range(16)]

            # ---------------- projections ----------------
            with tc.tile_pool(name="xw", bufs=1) as xw, \
                 tc.tile_pool(name="pps", bufs=4, space="PSUM") as pps:
                xkv = [xw.tile([P, NK], DT, tag=f"xkv{d}", name=f"xkv{d}") for d in range(8)]
                xq = [xw.tile([P, NQ], DT, tag=f"xq{d}", name=f"xq{d}") for d in range(8)]
                wq = [xw.tile([P, 1024], DT, tag=f"wq{d}", name=f"wq{d}") for d in range(8)]
                wk = [xw.tile([P, 1024], DT, tag=f"wk{d}", name=f"wk{d}") for d in range(8)]
                wv = [xw.tile([P, 1024], DT, tag=f"wv{d}", name=f"wv{d}") for d in range(8)]
                for d in range(8):
                    r = slice(d * P, (d + 1) * P)
                    nc.sync.dma_start(out=xkv[d][:], in_=xkv_d[r, :])
                    nc.sync.dma_start(out=xq[d][:], in_=xq_d[r, :])
                    nc.sync.dma_start(out=wq[d][:], in_=wq_d[r, :])
                    nc.sync.dma_start(out=wk[d][:], in_=wk_d[r, :])
                    nc.sync.dma_start(out=wv[d][:], in_=wv_d[r, :])

                # Q^T[dk, q] (pre-scaled): lhsT = wqT[d, dk-blk], rhs = xT_q[d, q-chunk]
                for dk in range(8):
                    wcol = slice(dk * P, (dk + 1) * P)
                    for qc in range(2):
                        cs = slice(qc * C, (qc + 1) * C)
                        ps = pps.tile([P, C], F32, tag="pps", name=f"psq{dk}_{qc}")
                        for d in range(8):
                            nc.tensor.matmul(ps[:], wq[d][:, wcol], xq[d][:, cs],
                                             start=(d == 0), stop=(d == 7))
                        nc.scalar.mul(QT[dk][:, cs], ps[:], SCALE)
                # K^T[dk, s]: lhsT = wkT[d, dk-blk], rhs = xT_kv[d, s-chunk]
                for dk in range(8):
                    wcol = slice(dk * P, (dk + 1) * P)
                    for sc in range(4):
                        cs = slice(sc * C, (sc + 1) * C)
                        ps = pps.tile([P, C], F32, tag="pps", name=f"psk{dk}_{sc}")
                        for d in range(8):
                            nc.tensor.matmul(ps[:], wk[d][:, wcol], xkv[d][:, cs],
                                             start=(d == 0), stop=(d == 7))
                        nc.vector.tensor_copy(KT[dk][:, cs], ps[:])
                # V[s, v]: lhsT = xT_kv[d, s-blk], rhs = wvT[d, v-chunk]
                for st in range(16):
                    xcol = slice(st * P, (st + 1) * P)
                    for vc in range(2):
                        cs = slice(vc * C, (vc + 1) * C)
                        ps = pps.tile([P, C], F32, tag="pps", name=f"psv{st}_{vc}")
                        for d in range(8):
                            nc.tensor.matmul(ps[:], xkv[d][:, xcol], wv[d][:, cs],
                                             start=(d == 0), stop=(d == 7))
                        nc.scalar.copy(V[st][:, cs], ps[:])

            # ---------------- attention ----------------
            with tc.tile_pool(name="attn", bufs=2) as at, \
                 tc.tile_pool(name="pts", bufs=2) as ptp, \
                 tc.tile_pool(name="stat", bufs=3) as stat, \
                 tc.tile_pool(name="con", bufs=1) as con, \
                 tc.tile_pool(name="sps", bufs=2, space="PSUM") as sps, \
                 tc.tile_pool(name="tps", bufs=2, space="PSUM") as tps, \
                 tc.tile_pool(name="yps", bufs=4, space="PSUM") as yps:
                ident = con.tile([P, P], DT, tag="ident", name="ident")
                make_identity(nc, ident[:])
                for blk in range(NB):
                    qs = slice(blk * P, (blk + 1) * P)
                    mask = at.tile([P, NK], F32, tag="mask", name=f"mask{blk}")
                    nc.sync.dma_start(out=mask[:], in_=mask_d[blk])
                    s_sb = at.tile([P, NK], F32, tag="s_sb", name=f"s_sb{blk}")
                    for sc in range(4):
                        cs = slice(sc * C, (sc + 1) * C)
                        ps = sps.tile([P, C], F32, tag="sps", name=f"sps{blk}_{sc}")
                        for dk in range(8):
                            nc.tensor.matmul(ps[:], QT[dk][:, qs], KT[dk][:, cs],
                                             start=(dk == 0), stop=(dk == 7))
                        nc.vector.tensor_tensor(out=s_sb[:, cs], in0=ps[:],
                                                in1=mask[:, cs],
                                                op=mybir.AluOpType.add)
                    negmax = stat.tile([P, 1], F32, tag="negmax", name=f"nm{blk}")
                    nc.vector.reduce_max(negmax[:], s_sb[:],
                                         axis=mybir.AxisListType.X, negate=True)
                    p_sb = at.tile([P, NK], DT, tag="p_sb", name=f"p_sb{blk}")
                    den = stat.tile([P, 1], F32, tag="den", name=f"den{blk}")
                    nc.scalar.activation(p_sb[:], s_sb[:],
                                         mybir.ActivationFunctionType.Exp,
                                         bias=negmax[:], scale=1.0,
                                         accum_out=den[:])
                    rec = stat.tile([P, 1], F32, tag="rec", name=f"rec{blk}")
                    nc.vector.reciprocal(rec[:], den[:])
                    pts = []
                    for st in range(16):
                        ss = slice(st * P, (st + 1) * P)
                        tp = tps.tile([P, P], DT, tag="tps", name=f"tp{blk}_{st}")
                        nc.tensor.transpose(tp[:], p_sb[:, ss], ident[:])
                        pt = ptp.tile([P, P], DT, tag=f"pt{st}", name=f"pt{blk}_{st}")
                        nc.vector.tensor_copy(pt[:], tp[:])
                        pts.append(pt)
                    for vc in range(2):
                        cs = slice(vc * C, (vc + 1) * C)
                        yp = yps.tile([P, C], F32, tag="yps", name=f"yp{blk}_{vc}")
                        for st in range(16):
                            nc.tensor.matmul(yp[:], pts[st][:], V[st][:, cs],
                                             start=(st == 0), stop=(st == 15))
                        y_sb = at.tile([P, C], F32, tag="y_sb", name=f"ysb{blk}_{vc}")
                        nc.scalar.activation(y_sb[:], yp[:],
                                             mybir.ActivationFunctionType.Copy,
                                             bias=0.0, scale=rec[:])
                        nc.sync.dma_start(out=y_d[blk, :, cs], in_=y_sb[:])

    _split_multi_waits(nc)
    return nc


def _host_inputs(x, Wq, Wk, Wv):
    np_dt = ml_dtypes.bfloat16
    wqT = np.ascontiguousarray(np.asarray(Wq, np.float32).T).astype(np_dt)
    wkT = np.ascontiguousarray(np.asarray(Wk, np.float32).T).astype(np_dt)
    wvT = np.ascontiguousarray(np.asarray(Wv, np.float32).T).astype(np_dt)
    col = np.arange(NK)[None, :]
    row = np.arange(P)[:, None]
    mask_s = []
    for s in range(2):
        m = np.empty((NB, P, NK), np.float32)
        for j in range(NB):
            g0 = s * 1024 + j * P
            m[j] = np.where(col <= (g0 + row), 0.0, -1e9)
        mask_s.append(m)
    ins = []
    for c in range(8):
        b, s = c // 2, c % 2
        xb = np.asarray(x[b], dtype=np.float32)
        q0 = s * 1024
        ins.append({
            "xkv": np.ascontiguousarray(xb.T).astype(np_dt),
            "xq": np.ascontiguousarray(xb[q0:q0 + 1024].T).astype(np_dt),
            "wq": wqT, "wk": wkT, "wv": wvT,
            "masks": mask_s[s],
        })
    return ins


def kernel(x, Wq, Wk, Wv):
    nc = _build()
    ins = _host_inputs(x, Wq, Wk, Wv)
    res = run_bass_kernel_spmd(nc, ins, list(range(8))).results
    y = np.empty((B, N, DV), np.float32)
    for c in range(8):
        b, s = c // 2, c % 2
        y[b, s * 1024:(s + 1) * 1024] = res[c]["y"].reshape(1024, 1024)
    return y
